# revision 5
# baseline (speedup 1.0000x reference)
"""Trainium2 Bass kernel for nn_ContrastiveLoss (binary-label supervised
contrastive loss over an 8192x8192 cosine-similarity matrix).

Math: with binary targets, each sample has class q = 2*tt + tp in {0..3}.
pos_mask(i,j) <=> class(j) == q_i^1, neg_mask(i,j) <=> class(j) == q_i^2, so
rows of classes {0,3} only need columns of classes {1,2} and vice versa.
Per row: loss_i = valid_i * (y_i.S_pos/(T*pos_cnt) - log(sum_j exp(sim_ij/T)))
where the j-sum runs over the two needed classes.

Device does the O(B^2) part: row-block x col-block dots (PE), exp + row-sum
split across the Scalar engine (native Exp+accum) and the Vector engine (two
custom DVE ops: deg-4 poly p~=exp(t/16), then p^16 with fused accumulate).
Host does O(B*D) prep (normalize, transpose, bf16 cast, class packing) and
the O(B) finalization (log, pos-term, masking).

Sharding: cores 0-3 take {0,3}-class anchor rows, cores 4-7 take {1,2}; each
core computes 9 chunks x NJe cols of exp-sums and returns 54 partial-sum
columns; host combines.
"""

import sys

if "/opt/trn_rl_repo" not in sys.path:
    sys.path.insert(0, "/opt/trn_rl_repo")

from contextlib import ExitStack
from operator import add

import numpy as np
import ml_dtypes

import concourse.bass as bass
import concourse.bacc as bacc
import concourse.tile as tile
from concourse import mybir
from concourse.bass_utils import run_bass_kernel_spmd
from concourse.dve_ops import (DveOp, OPS, CUSTOM_DVE_SPECS,
                               _SUB_OPCODE_FOR_NAME, _CUSTOM_DVE_ROW_BASE)
from concourse.dve_spec import (Spec, Src0, Src1, C0, C1, C2, C3, One, sq,
                                lower as dve_lower, _has_src1,
                                _spill_c3_to_src1)
from concourse.dve_uop import DveOpSpec

F32 = mybir.dt.float32
BF16 = mybir.dt.bfloat16
AF = mybir.ActivationFunctionType

B, D = 8192, 128
TEMP = 0.1
N_CORES = 8
F_CHUNKS = 9                # 9 f-chunks of 128 rows per core (capacity 1152)
FP = F_CHUNKS * 128
G0, G1 = 1536, 1536         # first two dots groups (3 PSUM banks each)

# deg-4 fit of e^u on [-0.625, 0.625] (a0=1), in x=sim: p=1+x(b1+x(b2+x(b3+x b4)))
PB = (0.6247442364692688, 0.1953597515821457,
      0.041675373911857605, 0.006334216333925724)

_program_cache = {}


def _register(name, spec):
    if name in _SUB_OPCODE_FOR_NAME:
        return next(op for op in OPS if op.name == name)
    row = _CUSTOM_DVE_ROW_BASE + len(OPS)
    assert row < 0x20
    _SUB_OPCODE_FOR_NAME[name] = row
    shas = {}
    for ver in ("v3", "v4"):
        uops = dve_lower(spec, ver=ver)
        shas[ver] = DveOpSpec(name=name, opcode=row, uops=uops,
                              rd1_en=_has_src1(spec)).sha(ver)
    op = DveOp(name, spec, subdim=False, uops_sha=shas)
    OPS.append(op)
    CUSTOM_DVE_SPECS[name] = spec
    return op


def _exp_ops():
    body = One + Src0 * (C0 + Src0 * (C1 + Src0 * (C2 + Src0 * C3)))
    p_ref = lambda in0, in1, s0, s1, imm2: (
        1 + in0 * (s0 + in0 * (s1 + in0 * (imm2 + in0 * in1)))).astype(np.float32)
    op_poly = _register("EXPQ16_POLY_ANT",
                        Spec(body=_spill_c3_to_src1(body), reference=p_ref))

    def pow16_ref(in0, in1, s0, s1, imm2):
        b = (in0.astype(np.float32) ** 16).astype(np.float32)
        return b, b.reshape(b.shape[0], -1).sum(-1, keepdims=True)

    op_pow = _register("POW16_ACC_ANT",
                       Spec(body=sq(sq(sq(sq(Src0)))), accum=add,
                            reference=pow16_ref))
    return op_poly, op_pow


def _dve_split(gw):
    """Columns the DVE engine takes from the FRONT of a gw-wide group
    (balances ACT 1/1.2GHz+reads vs DVE 2cpe/0.96GHz+overheads)."""
    w = ((gw + 172) / 1.2 - 32) / 2.916
    return max(0, min(gw, int(w) & ~1))


def build_program(NJe: int):
    op_poly, op_pow = _exp_ops()
    nc = bacc.Bacc("TRN2", target_bir_lowering=False, debug=False,
                   num_devices=N_CORES)

    ytf = nc.declare_dram_parameter("ytf", [128, FP], BF16, isOutput=False)
    ytj = nc.declare_dram_parameter("ytj", [128, NJe], BF16, isOutput=False)
    acc_out = nc.declare_dram_parameter("acc", [128, 6 * F_CHUNKS], F32,
                                        isOutput=True)

    gw0 = (NJe // 3 + 1) & ~1
    groups = [(0, gw0), (gw0, gw0), (2 * gw0, NJe - 2 * gw0)]
    assert 512 < groups[2][1] <= 1536

    with ExitStack() as ctx:
        tc = ctx.enter_context(tile.TileContext(nc))
        persist = ctx.enter_context(tc.tile_pool(name="persist", bufs=1))
        pqpool = ctx.enter_context(tc.tile_pool(name="pq", bufs=2))
        dots_ps = ctx.enter_context(tc.tile_pool(name="dots", bufs=2,
                                                 space="PSUM"))

        A = persist.tile([128, F_CHUNKS, 3, 2], F32)
        c3t = persist.tile([128, 1], F32)
        nc.vector.memset(c3t, float(PB[3]))
        es = persist.tile([128, 1152], BF16)    # ACT exp dump (discarded)
        pd = persist.tile([128, 704], BF16)     # op2 dump (discarded)
        warm = persist.tile([128, 1], BF16)
        nc.scalar.activation(out=warm, in_=c3t, func=AF.Exp)  # table preload

        YTf = persist.tile([128, FP], BF16)
        YTj = persist.tile([128, NJe], BF16)
        nc.sync.dma_start(out=YTf, in_=ytf[:])
        nc.gpsimd.dma_start(out=YTj[:, 0:gw0], in_=ytj[:, 0:gw0])
        nc.scalar.dma_start(out=YTj[:, gw0:2 * gw0], in_=ytj[:, gw0:2 * gw0])
        nc.sync.dma_start(out=YTj[:, 2 * gw0:NJe], in_=ytj[:, 2 * gw0:NJe])

        for gi, (j0, gw) in enumerate(groups):
            wd = _dve_split(gw)
            for c in range(F_CHUNKS):
                lhsT = YTf[:, c * 128:(c + 1) * 128]
                dp = dots_ps.tile([128, 1536], F32, tag="dots")
                b0 = 0
                while b0 < gw:
                    bw = min(512, gw - b0)
                    nc.tensor.matmul(dp[:, b0:b0 + bw], lhsT=lhsT,
                                     rhs=YTj[:, j0 + b0:j0 + b0 + bw],
                                     start=True, stop=True)
                    b0 += bw
                pq = pqpool.tile([128, 704], BF16, tag="pq")
                nc.vector._custom_dve(op_poly, out=pq[:, :wd],
                                      in0=dp[:, :wd], in1=c3t,
                                      s0=float(PB[0]), s1=float(PB[1]),
                                      imm2=float(PB[2]))
                nc.vector._custom_dve(op_pow, out=pd[:, :wd],
                                      in0=pq[:, :wd],
                                      accum_out=A[:, c, gi, 1:2])
                nc.scalar.activation(out=es[:, :gw - wd], in_=dp[:, wd:gw],
                                     func=AF.Exp, scale=1.0 / TEMP,
                                     accum_out=A[:, c, gi, 0:1])

        nc.sync.dma_start(out=acc_out[:], in_=A)

    nc.compile()
    return nc


def host_shard(features, data_ix, targets_t, targets_p):
    tt = np.asarray(targets_t)[np.asarray(data_ix)].astype(np.int32)
    tp = np.asarray(targets_p)[np.asarray(data_ix)].astype(np.int32)
    q = 2 * tt + tp
    cnt = np.bincount(q, minlength=4)
    pos_cnt = cnt[q ^ 1]
    neg_cnt = cnt[q ^ 2]
    valid = (pos_cnt > 0) & (neg_cnt > 0)

    feats = np.asarray(features, np.float32)
    norms = np.sqrt((feats * feats).sum(1))
    y = feats / np.maximum(norms, 1e-8)[:, None]
    ybf = y.astype(ml_dtypes.bfloat16)

    idx = [np.nonzero(q == c)[0] for c in range(4)]
    a_rows = np.concatenate([idx[0], idx[3]])      # cores 0-3
    b_rows = np.concatenate([idx[1], idx[2]])      # cores 4-7
    assert len(a_rows) <= 4 * FP and len(b_rows) <= 4 * FP

    W1 = (max(len(idx[1]), len(idx[0])) + 1) & ~1
    W2 = (max(len(idx[2]), len(idx[3])) + 1) & ~1
    NJe = W1 + W2
    if NJe - G0 - G1 <= 512:          # keep last group in (512, 1536]
        NJe = G0 + G1 + 514
    npad = [NJe - cnt[1] - cnt[2], NJe - cnt[0] - cnt[3]]

    def jside(c1, c2):
        out = np.zeros((128, NJe), ml_dtypes.bfloat16)
        out[:, :len(idx[c1])] = ybf[idx[c1]].T
        out[:, W1:W1 + len(idx[c2])] = ybf[idx[c2]].T
        return out

    ytj_sides = [jside(1, 2), jside(0, 3)]

    in_maps, core_rows = [], []
    for k in range(N_CORES):
        side = 0 if k < 4 else 1
        rows = (a_rows if side == 0 else b_rows)[k % 4 * FP:(k % 4 + 1) * FP]
        ytf = np.zeros((128, FP), ml_dtypes.bfloat16)
        ytf[:, :len(rows)] = ybf[rows].T
        in_maps.append({"ytf": ytf, "ytj": ytj_sides[side]})
        core_rows.append(rows)
    meta = dict(q=q, pos_cnt=pos_cnt, valid=valid, y=y, idx=idx, npad=npad,
                core_rows=core_rows)
    return in_maps, NJe, meta


def finalize(results, meta):
    q, pos_cnt, valid, y = meta["q"], meta["pos_cnt"], meta["valid"], meta["y"]
    denom = np.zeros(B, np.float64)
    for k, r in enumerate(results):
        rows = meta["core_rows"][k]
        a = np.asarray(r["acc"], np.float64).reshape(128, F_CHUNKS, 6)
        per_row = a.sum(2).T.reshape(-1)            # [FP] chunk-major rows
        side = 0 if k < 4 else 1
        denom[rows] = per_row[:len(rows)] - meta["npad"][side]
    S = np.stack([y[meta["idx"][c]].sum(0) for c in range(4)])   # [4, D]
    LS = (y @ S.T)[np.arange(B), q ^ 1]
    log_denom = np.log(np.maximum(denom, 1e-300))
    mlp = np.where(valid, LS / (TEMP * np.maximum(pos_cnt, 1)) - log_denom, 0.0)
    return np.float32(-mlp.sum() / B)


def run_on_device(in_maps, NJe, **kw):
    if NJe not in _program_cache:
        _program_cache[NJe] = build_program(NJe)
    return run_bass_kernel_spmd(_program_cache[NJe], in_maps,
                                list(range(N_CORES)), **kw)


def kernel(features, data_ix, targets_t, targets_p):
    in_maps, NJe, meta = host_shard(features, data_ix, targets_t, targets_p)
    res = run_on_device(in_maps, NJe)
    return finalize(res.results, meta)


if __name__ == "__main__":
    import importlib.util

    spec = importlib.util.spec_from_file_location(
        "reference", "/root/problem/reference.py")
    ref = importlib.util.module_from_spec(spec)
    spec.loader.exec_module(ref)
    inputs = {k: np.asarray(v) for k, v in ref.setup_inputs().items()}
    out = kernel(**inputs)
    print("kernel loss:", out)


# revision 7
# speedup vs baseline: 1.0799x; 1.0799x over previous
"""Trainium2 Bass kernel for nn_ContrastiveLoss (binary-label supervised
contrastive loss over an 8192x8192 cosine-similarity matrix).

Math: with binary targets, each sample has class q = 2*tt + tp in {0..3}.
pos_mask(i,j) <=> class(j) == q_i^1, neg_mask(i,j) <=> class(j) == q_i^2, so
rows of classes {0,3} only need columns of classes {1,2} and vice versa.
Per row: loss_i = valid_i * (y_i.S_pos/(T*pos_cnt) - log(sum_j exp(sim_ij/T)))
where the j-sum runs over the two needed classes.

Device does the O(B^2) part: row-block x col-block dots (PE), exp + row-sum
split across the Scalar engine (native Exp+accum) and the Vector engine (two
custom DVE ops: deg-4 poly p~=exp(t/16), then p^16 with fused accumulate).
Host does O(B*D) prep (normalize, transpose, bf16 cast, class packing) and
the O(B) finalization (log, pos-term, masking).

Sharding: cores 0-3 take {0,3}-class anchor rows, cores 4-7 take {1,2}; each
core computes 9 chunks x NJe cols of exp-sums and returns 54 partial-sum
columns; host combines.
"""

import sys

if "/opt/trn_rl_repo" not in sys.path:
    sys.path.insert(0, "/opt/trn_rl_repo")

from contextlib import ExitStack
from operator import add

import numpy as np
import ml_dtypes

import concourse.bass as bass
import concourse.bacc as bacc
import concourse.tile as tile
from concourse import mybir
from concourse.bass_utils import run_bass_kernel_spmd
from concourse.dve_ops import (DveOp, OPS, CUSTOM_DVE_SPECS,
                               _SUB_OPCODE_FOR_NAME, _CUSTOM_DVE_ROW_BASE)
from concourse.dve_spec import (Spec, Src0, Src1, C0, C1, C2, C3, One, sq,
                                lower as dve_lower, _has_src1,
                                _spill_c3_to_src1)
from concourse.dve_uop import DveOpSpec

F32 = mybir.dt.float32
BF16 = mybir.dt.bfloat16
AF = mybir.ActivationFunctionType

B, D = 8192, 128
TEMP = 0.1
N_CORES = 8
F_CHUNKS = 9                # 9 f-chunks of 128 rows per core (capacity 1152)
FP = F_CHUNKS * 128
G0, G1 = 1536, 1536         # first two dots groups (3 PSUM banks each)

# deg-4 fit of e^u on [-0.625, 0.625] (a0=1), in x=sim: p=1+x(b1+x(b2+x(b3+x b4)))
PB = (0.6247442364692688, 0.1953597515821457,
      0.041675373911857605, 0.006334216333925724)

_program_cache = {}


def _register(name, spec):
    if name in _SUB_OPCODE_FOR_NAME:
        return next(op for op in OPS if op.name == name)
    row = _CUSTOM_DVE_ROW_BASE + len(OPS)
    assert row < 0x20
    _SUB_OPCODE_FOR_NAME[name] = row
    shas = {}
    for ver in ("v3", "v4"):
        uops = dve_lower(spec, ver=ver)
        shas[ver] = DveOpSpec(name=name, opcode=row, uops=uops,
                              rd1_en=_has_src1(spec)).sha(ver)
    op = DveOp(name, spec, subdim=False, uops_sha=shas)
    OPS.append(op)
    CUSTOM_DVE_SPECS[name] = spec
    return op


def _exp_ops():
    body = One + Src0 * (C0 + Src0 * (C1 + Src0 * (C2 + Src0 * C3)))
    p_ref = lambda in0, in1, s0, s1, imm2: (
        1 + in0 * (s0 + in0 * (s1 + in0 * (imm2 + in0 * in1)))).astype(np.float32)
    op_poly = _register("EXPQ16_POLY_ANT",
                        Spec(body=_spill_c3_to_src1(body), reference=p_ref))

    def pow16_ref(in0, in1, s0, s1, imm2):
        b = (in0.astype(np.float32) ** 16).astype(np.float32)
        return b, b.reshape(b.shape[0], -1).sum(-1, keepdims=True)

    op_pow = _register("POW16_ACC_ANT",
                       Spec(body=sq(sq(sq(sq(Src0)))), accum=add,
                            reference=pow16_ref))
    return op_poly, op_pow


def _dve_split(gw):
    """Columns the DVE engine takes from the FRONT of a gw-wide group
    (balances ACT 1/1.2GHz+reads vs DVE 2cpe/0.96GHz+overheads)."""
    w = ((gw + 172) / 1.2 - 32) / 2.916
    return max(0, min(gw, int(w) & ~1))


def build_program(NJe: int):
    op_poly, op_pow = _exp_ops()
    nc = bacc.Bacc("TRN2", target_bir_lowering=False, debug=False,
                   num_devices=N_CORES)

    ytf = nc.declare_dram_parameter("ytf", [128, FP], BF16, isOutput=False)
    ytj = nc.declare_dram_parameter("ytj", [128, NJe], BF16, isOutput=False)
    acc_out = nc.declare_dram_parameter("acc", [128, 6 * F_CHUNKS], F32,
                                        isOutput=True)

    gw0 = (NJe // 3 + 1) & ~1
    groups = [(0, gw0), (gw0, gw0), (2 * gw0, NJe - 2 * gw0)]
    for _, gw in groups:
        assert 512 < gw <= 1536 and _dve_split(gw) <= 512 \
            and gw - _dve_split(gw) <= 1024

    with ExitStack() as ctx:
        tc = ctx.enter_context(tile.TileContext(nc))
        persist = ctx.enter_context(tc.tile_pool(name="persist", bufs=1))
        pqpool = ctx.enter_context(tc.tile_pool(name="pq", bufs=2))
        dd_ps = ctx.enter_context(tc.tile_pool(name="dd", bufs=2,
                                               space="PSUM"))
        da_ps = ctx.enter_context(tc.tile_pool(name="da", bufs=3,
                                               space="PSUM"))

        Aact = persist.tile([128, 3 * F_CHUNKS], F32)
        Adve = persist.tile([128, 3 * F_CHUNKS], F32)
        c3t = persist.tile([128, 1], F32)
        nc.vector.memset(c3t, float(PB[3]))
        es = persist.tile([128, 1024], BF16)    # ACT exp dump (discarded)
        pd = persist.tile([128, 512], BF16)     # op2 dump (discarded)
        warm = persist.tile([128, 1], BF16)
        nc.scalar.activation(out=warm, in_=c3t, func=AF.Exp)  # table preload

        YTf = persist.tile([128, FP], BF16)
        YTj = persist.tile([128, NJe], BF16)
        nc.sync.dma_start(out=YTf, in_=ytf[:])
        nc.sync.dma_start(out=YTj[:, 0:gw0], in_=ytj[:, 0:gw0])
        nc.sync.dma_start(out=YTj[:, gw0:2 * gw0], in_=ytj[:, gw0:2 * gw0])
        nc.sync.dma_start(out=YTj[:, 2 * gw0:NJe], in_=ytj[:, 2 * gw0:NJe])

        for gi, (j0, gw) in enumerate(groups):
            wd = _dve_split(gw)
            wa = gw - wd
            for c in range(F_CHUNKS):
                lhsT = YTf[:, c * 128:(c + 1) * 128]
                dd = dd_ps.tile([128, 512], F32, tag="dd")
                nc.tensor.matmul(dd[:, :wd], lhsT=lhsT,
                                 rhs=YTj[:, j0:j0 + wd],
                                 start=True, stop=True)
                da = da_ps.tile([128, 1024], F32, tag="da")
                b0 = 0
                while b0 < wa:
                    bw = min(512, wa - b0)
                    nc.tensor.matmul(da[:, b0:b0 + bw], lhsT=lhsT,
                                     rhs=YTj[:, j0 + wd + b0:j0 + wd + b0 + bw],
                                     start=True, stop=True)
                    b0 += bw
                pq = pqpool.tile([128, 512], BF16, tag="pq")
                nc.vector._custom_dve(op_poly, out=pq[:, :wd],
                                      in0=dd[:, :wd], in1=c3t,
                                      s0=float(PB[0]), s1=float(PB[1]),
                                      imm2=float(PB[2]))
                nc.vector._custom_dve(op_pow, out=pd[:, :wd],
                                      in0=pq[:, :wd],
                                      accum_out=Adve[:, 3 * c + gi:3 * c + gi + 1])
                nc.scalar.activation(out=es[:, :wa], in_=da[:, :wa],
                                     func=AF.Exp, scale=1.0 / TEMP,
                                     accum_out=Aact[:, 3 * c + gi:3 * c + gi + 1])

        nc.sync.dma_start(out=acc_out[:, 0:27], in_=Aact)
        nc.sync.dma_start(out=acc_out[:, 27:54], in_=Adve)

    nc.compile()
    return nc


def host_shard(features, data_ix, targets_t, targets_p):
    tt = np.asarray(targets_t)[np.asarray(data_ix)].astype(np.int32)
    tp = np.asarray(targets_p)[np.asarray(data_ix)].astype(np.int32)
    q = 2 * tt + tp
    cnt = np.bincount(q, minlength=4)
    pos_cnt = cnt[q ^ 1]
    neg_cnt = cnt[q ^ 2]
    valid = (pos_cnt > 0) & (neg_cnt > 0)

    feats = np.asarray(features, np.float32)
    norms = np.sqrt((feats * feats).sum(1))
    y = feats / np.maximum(norms, 1e-8)[:, None]
    ybf = y.astype(ml_dtypes.bfloat16)

    idx = [np.nonzero(q == c)[0] for c in range(4)]
    a_rows = np.concatenate([idx[0], idx[3]])      # cores 0-3
    b_rows = np.concatenate([idx[1], idx[2]])      # cores 4-7
    assert len(a_rows) <= 4 * FP and len(b_rows) <= 4 * FP

    W1 = (max(len(idx[1]), len(idx[0])) + 1) & ~1
    W2 = (max(len(idx[2]), len(idx[3])) + 1) & ~1
    NJe = W1 + W2
    if NJe - G0 - G1 <= 512:          # keep last group in (512, 1536]
        NJe = G0 + G1 + 514
    npad = [NJe - cnt[1] - cnt[2], NJe - cnt[0] - cnt[3]]

    def jside(c1, c2):
        out = np.zeros((128, NJe), ml_dtypes.bfloat16)
        out[:, :len(idx[c1])] = ybf[idx[c1]].T
        out[:, W1:W1 + len(idx[c2])] = ybf[idx[c2]].T
        return out

    ytj_sides = [jside(1, 2), jside(0, 3)]

    in_maps, core_rows = [], []
    for k in range(N_CORES):
        side = 0 if k < 4 else 1
        rows = (a_rows if side == 0 else b_rows)[k % 4 * FP:(k % 4 + 1) * FP]
        ytf = np.zeros((128, FP), ml_dtypes.bfloat16)
        ytf[:, :len(rows)] = ybf[rows].T
        in_maps.append({"ytf": ytf, "ytj": ytj_sides[side]})
        core_rows.append(rows)
    meta = dict(q=q, pos_cnt=pos_cnt, valid=valid, y=y, idx=idx, npad=npad,
                core_rows=core_rows)
    return in_maps, NJe, meta


def finalize(results, meta):
    q, pos_cnt, valid, y = meta["q"], meta["pos_cnt"], meta["valid"], meta["y"]
    denom = np.zeros(B, np.float64)
    for k, r in enumerate(results):
        rows = meta["core_rows"][k]
        a = np.asarray(r["acc"], np.float64).reshape(128, 2, F_CHUNKS, 3)
        per_row = a.sum((1, 3)).T.reshape(-1)       # [FP] chunk-major rows
        side = 0 if k < 4 else 1
        denom[rows] = per_row[:len(rows)] - meta["npad"][side]
    S = np.stack([y[meta["idx"][c]].sum(0) for c in range(4)])   # [4, D]
    LS = (y @ S.T)[np.arange(B), q ^ 1]
    log_denom = np.log(np.maximum(denom, 1e-300))
    mlp = np.where(valid, LS / (TEMP * np.maximum(pos_cnt, 1)) - log_denom, 0.0)
    return np.float32(-mlp.sum() / B)


def run_on_device(in_maps, NJe, **kw):
    if NJe not in _program_cache:
        _program_cache[NJe] = build_program(NJe)
    return run_bass_kernel_spmd(_program_cache[NJe], in_maps,
                                list(range(N_CORES)), **kw)


def kernel(features, data_ix, targets_t, targets_p):
    in_maps, NJe, meta = host_shard(features, data_ix, targets_t, targets_p)
    res = run_on_device(in_maps, NJe)
    return finalize(res.results, meta)


if __name__ == "__main__":
    import importlib.util

    spec = importlib.util.spec_from_file_location(
        "reference", "/root/problem/reference.py")
    ref = importlib.util.module_from_spec(spec)
    spec.loader.exec_module(ref)
    inputs = {k: np.asarray(v) for k, v in ref.setup_inputs().items()}
    out = kernel(**inputs)
    print("kernel loss:", out)


# revision 11
# speedup vs baseline: 1.2511x; 1.1586x over previous
"""Trainium2 Bass kernel for nn_ContrastiveLoss (binary-label supervised
contrastive loss over an 8192x8192 cosine-similarity matrix).

Math: with binary targets, each sample has class q = 2*tt + tp in {0..3}.
pos_mask(i,j) <=> class(j) == q_i^1, neg_mask(i,j) <=> class(j) == q_i^2, so
rows of classes {0,3} only need columns of classes {1,2} and vice versa.
Per row: loss_i = valid_i * (y_i.S_pos/(T*pos_cnt) - log(sum_j exp(sim_ij/T)))
where the j-sum runs over the two needed classes.

Device does the O(B^2) part: row-block x col-block dots (PE), exp + row-sum
split across the Scalar engine (native Exp+accum) and the Vector engine (two
custom DVE ops: deg-4 poly p~=exp(t/16), then p^16 with fused accumulate).
Host does O(B*D) prep (normalize, transpose, bf16 cast, class packing) and
the O(B) finalization (log, pos-term, masking).

Sharding: cores 0-3 take {0,3}-class anchor rows, cores 4-7 take {1,2}; each
core computes 9 chunks x NJe cols of exp-sums and returns 54 partial-sum
columns; host combines.
"""

import sys

if "/opt/trn_rl_repo" not in sys.path:
    sys.path.insert(0, "/opt/trn_rl_repo")

from contextlib import ExitStack
from operator import add

import numpy as np
import ml_dtypes

import concourse.bass as bass
import concourse.bacc as bacc
import concourse.tile as tile
from concourse import mybir
from concourse.bass_utils import run_bass_kernel_spmd
from concourse.dve_ops import (DveOp, OPS, CUSTOM_DVE_SPECS,
                               _SUB_OPCODE_FOR_NAME, _CUSTOM_DVE_ROW_BASE)
from concourse.dve_spec import (Spec, Src0, Src1, C0, C1, C2, C3, One, sq,
                                lower as dve_lower, _has_src1,
                                _spill_c3_to_src1)
from concourse.dve_uop import DveOpSpec

F32 = mybir.dt.float32
BF16 = mybir.dt.bfloat16
AF = mybir.ActivationFunctionType

B, D = 8192, 128
TEMP = 0.1
N_CORES = 8
F_CHUNKS = 9                # 9 f-chunks of 128 rows per core (capacity 1152)
FP = F_CHUNKS * 128
G0, G1 = 1536, 1536         # first two dots groups (3 PSUM banks each)

# deg-4 fit of e^u on [-0.625, 0.625] (a0=1), in x=sim: p=1+x(b1+x(b2+x(b3+x b4)))
PB = (0.6247442364692688, 0.1953597515821457,
      0.041675373911857605, 0.006334216333925724)

_program_cache = {}


def _register(name, spec):
    if name in _SUB_OPCODE_FOR_NAME:
        return next(op for op in OPS if op.name == name)
    row = _CUSTOM_DVE_ROW_BASE + len(OPS)
    assert row < 0x20
    _SUB_OPCODE_FOR_NAME[name] = row
    shas = {}
    for ver in ("v3", "v4"):
        uops = dve_lower(spec, ver=ver)
        shas[ver] = DveOpSpec(name=name, opcode=row, uops=uops,
                              rd1_en=_has_src1(spec)).sha(ver)
    op = DveOp(name, spec, subdim=False, uops_sha=shas)
    OPS.append(op)
    CUSTOM_DVE_SPECS[name] = spec
    return op


def _exp_ops():
    body = One + Src0 * (C0 + Src0 * (C1 + Src0 * (C2 + Src0 * C3)))
    p_ref = lambda in0, in1, s0, s1, imm2: (
        1 + in0 * (s0 + in0 * (s1 + in0 * (imm2 + in0 * in1)))).astype(np.float32)
    op_poly = _register("EXPQ16_POLY_ANT",
                        Spec(body=_spill_c3_to_src1(body), reference=p_ref))

    def pow16_ref(in0, in1, s0, s1, imm2):
        b = (in0.astype(np.float32) ** 16).astype(np.float32)
        return b, b.reshape(b.shape[0], -1).sum(-1, keepdims=True)

    op_pow = _register("POW16_ACC_ANT",
                       Spec(body=sq(sq(sq(sq(Src0)))), accum=add,
                            reference=pow16_ref))
    return op_poly, op_pow


def _dve_split(gw):
    """Columns the DVE engine takes from the FRONT of a gw-wide group
    (balances ACT 1/1.2GHz+read-accum vs DVE ~2cpe with batched pow16)."""
    w = (0.833 * gw + 297) / 2.916
    return max(0, min(gw, int(w) & ~1))


def build_program(NJe: int):
    op_poly, op_pow = _exp_ops()
    nc = bacc.Bacc("TRN2", target_bir_lowering=False, debug=False,
                   num_devices=N_CORES)

    ytf = nc.declare_dram_parameter("ytf", [128, FP], BF16, isOutput=False)
    ytj = nc.declare_dram_parameter("ytj", [128, NJe], BF16, isOutput=False)
    acc_out = nc.declare_dram_parameter("acc", [128, 4 * F_CHUNKS], F32,
                                        isOutput=True)

    gw0 = (NJe // 3 + 1) & ~1
    groups = [(0, gw0), (gw0, gw0), (2 * gw0, NJe - 2 * gw0)]
    for _, gw in groups:
        assert 512 < gw <= 1536 and _dve_split(gw) <= 512 \
            and gw - _dve_split(gw) <= 1024

    with ExitStack() as ctx:
        tc = ctx.enter_context(tile.TileContext(nc))
        persist = ctx.enter_context(tc.tile_pool(name="persist", bufs=1))
        pqpool = ctx.enter_context(tc.tile_pool(name="pq", bufs=2))
        dd_ps = ctx.enter_context(tc.tile_pool(name="dd", bufs=2,
                                               space="PSUM"))
        da_ps = ctx.enter_context(tc.tile_pool(name="da", bufs=3,
                                               space="PSUM"))

        Aact = persist.tile([128, 3 * F_CHUNKS], F32)
        Adve = persist.tile([128, F_CHUNKS], F32)
        c3t = persist.tile([128, 1], F32)
        nc.vector.memset(c3t, float(PB[3]))
        es = persist.tile([128, 1024], BF16)    # ACT exp dump (discarded)
        pd = persist.tile([128, 1536], BF16)    # op2 dump (discarded)
        warm = persist.tile([128, 1], BF16)
        nc.scalar.activation(out=warm, in_=c3t, func=AF.Exp)  # table preload

        YTf = persist.tile([128, FP], BF16)
        YTj = persist.tile([128, NJe], BF16)
        nc.sync.dma_start(out=YTf, in_=ytf[:])
        nc.sync.dma_start(out=YTj[:, 0:gw0], in_=ytj[:, 0:gw0])
        nc.sync.dma_start(out=YTj[:, gw0:2 * gw0], in_=ytj[:, gw0:2 * gw0])
        nc.sync.dma_start(out=YTj[:, 2 * gw0:NJe], in_=ytj[:, 2 * gw0:NJe])

        wds = [_dve_split(gw) for _, gw in groups]
        WDT = sum(wds)
        for c in range(F_CHUNKS):
            lhsT = YTf[:, c * 128:(c + 1) * 128]
            pq = pqpool.tile([128, WDT], BF16, tag="pq")
            woff = 0
            for gi, (j0, gw) in enumerate(groups):
                wd = wds[gi]
                wa = gw - wd
                dd = dd_ps.tile([128, 512], F32, tag="dd")
                nc.tensor.matmul(dd[:, :wd], lhsT=lhsT,
                                 rhs=YTj[:, j0:j0 + wd],
                                 start=True, stop=True)
                da = da_ps.tile([128, 1024], F32, tag="da")
                b0 = 0
                while b0 < wa:
                    bw = min(512, wa - b0)
                    nc.tensor.matmul(da[:, b0:b0 + bw], lhsT=lhsT,
                                     rhs=YTj[:, j0 + wd + b0:j0 + wd + b0 + bw],
                                     start=True, stop=True)
                    b0 += bw
                nc.vector._custom_dve(op_poly, out=pq[:, woff:woff + wd],
                                      in0=dd[:, :wd], in1=c3t,
                                      s0=float(PB[0]), s1=float(PB[1]),
                                      imm2=float(PB[2]))
                nc.scalar.activation(out=es[:, :wa], in_=da[:, :wa],
                                     func=AF.Exp, scale=1.0 / TEMP,
                                     accum_out=Aact[:, 3 * c + gi:3 * c + gi + 1])
                woff += wd
            nc.vector._custom_dve(op_pow, out=pd[:, :WDT],
                                  in0=pq[:, :WDT],
                                  accum_out=Adve[:, c:c + 1])

        nc.sync.dma_start(out=acc_out[:, 0:27], in_=Aact)
        nc.sync.dma_start(out=acc_out[:, 27:36], in_=Adve)

    nc.compile()
    return nc


def host_shard(features, data_ix, targets_t, targets_p):
    tt = np.asarray(targets_t)[np.asarray(data_ix)].astype(np.int32)
    tp = np.asarray(targets_p)[np.asarray(data_ix)].astype(np.int32)
    q = 2 * tt + tp
    cnt = np.bincount(q, minlength=4)
    pos_cnt = cnt[q ^ 1]
    neg_cnt = cnt[q ^ 2]
    valid = (pos_cnt > 0) & (neg_cnt > 0)

    feats = np.asarray(features, np.float32)
    norms = np.sqrt((feats * feats).sum(1))
    y = feats / np.maximum(norms, 1e-8)[:, None]
    ybf = y.astype(ml_dtypes.bfloat16)

    idx = [np.nonzero(q == c)[0] for c in range(4)]
    a_rows = np.concatenate([idx[0], idx[3]])      # cores 0-3
    b_rows = np.concatenate([idx[1], idx[2]])      # cores 4-7
    assert len(a_rows) <= 4 * FP and len(b_rows) <= 4 * FP

    W1 = (max(len(idx[1]), len(idx[0])) + 1) & ~1
    W2 = (max(len(idx[2]), len(idx[3])) + 1) & ~1
    NJe = W1 + W2
    if NJe - G0 - G1 <= 512:          # keep last group in (512, 1536]
        NJe = G0 + G1 + 514
    npad = [NJe - cnt[1] - cnt[2], NJe - cnt[0] - cnt[3]]

    def jside(c1, c2):
        out = np.zeros((128, NJe), ml_dtypes.bfloat16)
        out[:, :len(idx[c1])] = ybf[idx[c1]].T
        out[:, W1:W1 + len(idx[c2])] = ybf[idx[c2]].T
        return out

    ytj_sides = [jside(1, 2), jside(0, 3)]

    in_maps, core_rows = [], []
    for k in range(N_CORES):
        side = 0 if k < 4 else 1
        rows = (a_rows if side == 0 else b_rows)[k % 4 * FP:(k % 4 + 1) * FP]
        ytf = np.zeros((128, FP), ml_dtypes.bfloat16)
        ytf[:, :len(rows)] = ybf[rows].T
        in_maps.append({"ytf": ytf, "ytj": ytj_sides[side]})
        core_rows.append(rows)
    meta = dict(q=q, pos_cnt=pos_cnt, valid=valid, y=y, idx=idx, npad=npad,
                core_rows=core_rows)
    return in_maps, NJe, meta


def finalize(results, meta):
    q, pos_cnt, valid, y = meta["q"], meta["pos_cnt"], meta["valid"], meta["y"]
    denom = np.zeros(B, np.float64)
    for k, r in enumerate(results):
        rows = meta["core_rows"][k]
        a = np.asarray(r["acc"], np.float64)
        per_chunk = a[:, :27].reshape(128, F_CHUNKS, 3).sum(2) + a[:, 27:36]
        per_row = per_chunk.T.reshape(-1)           # [FP] chunk-major rows
        side = 0 if k < 4 else 1
        denom[rows] = per_row[:len(rows)] - meta["npad"][side]
    S = np.stack([y[meta["idx"][c]].sum(0) for c in range(4)])   # [4, D]
    LS = (y @ S.T)[np.arange(B), q ^ 1]
    log_denom = np.log(np.maximum(denom, 1e-300))
    mlp = np.where(valid, LS / (TEMP * np.maximum(pos_cnt, 1)) - log_denom, 0.0)
    return np.float32(-mlp.sum() / B)


def run_on_device(in_maps, NJe, **kw):
    if NJe not in _program_cache:
        _program_cache[NJe] = build_program(NJe)
    return run_bass_kernel_spmd(_program_cache[NJe], in_maps,
                                list(range(N_CORES)), **kw)


def kernel(features, data_ix, targets_t, targets_p):
    in_maps, NJe, meta = host_shard(features, data_ix, targets_t, targets_p)
    res = run_on_device(in_maps, NJe)
    return finalize(res.results, meta)


if __name__ == "__main__":
    import importlib.util

    spec = importlib.util.spec_from_file_location(
        "reference", "/root/problem/reference.py")
    ref = importlib.util.module_from_spec(spec)
    spec.loader.exec_module(ref)
    inputs = {k: np.asarray(v) for k, v in ref.setup_inputs().items()}
    out = kernel(**inputs)
    print("kernel loss:", out)
